# revision 25
# baseline (speedup 1.0000x reference)
"""Trainium2 Bass kernel for nn_Decoder2 (dense transformer decoder block).

Sharding (8 cores):
  - both attentions: head-sharded, 2 heads (=128 feature dims) per core
  - FFN: hidden dim column/row sharded, 512 hidden units per core; the 8
    partial outputs are summed on the host
  - wemb/pemb replicated; all activations kept transposed [feat, seq]

The kernel is a software pipeline over 4 sequence chunks of 512: each
chunk's self-attention output is AllGathered independently, so the
collectives and the cross-attention/FFN for chunk c overlap later
attention chunks. Same for the cross->FFN boundary.

All matmul operands are bf16 (PE moving-operand rate is higher than
fp32r and DMA bytes halve); PSUM accumulation stays fp32, softmax
denominator/reciprocal stay fp32. Softmax is computed without
max-subtraction (scores are O(+-6)); the denominator comes from a
ones-column folded into the AV matmul (lhsT = [v_head | ones], m=65).
Scores for the two heads are issued adjacently as K=64 row-tiles
(tile_position) so they run concurrently on the PE.

Bulk weight prefetch rides the GpSimd DMA queue so the first wemb
chunk (Sync queue) isn't stuck behind it; w2 stays SBUF-resident.
"""

import ml_dtypes
import numpy as np

import concourse.bass as bass
import concourse.bacc as bacc
import concourse.mybir as mybir
import concourse.tile as tile
from concourse.bass_utils import run_bass_kernel_spmd
from concourse.masks import make_identity

F32 = mybir.dt.float32
BF16 = mybir.dt.bfloat16
AF = mybir.ActivationFunctionType

N_CORES = 8
S_W, S_P = 2048, 1024
D_MODEL, NEW_DIM, H, D_FF = 1024, 1024, 16, 4096
HD = 128          # head-feature dims per core (2 heads x 64)
FF_SH = D_FF // N_CORES   # 512 hidden units per core
NC = 512          # free-dim chunk for matmuls
DCH = D_MODEL // 128      # 8 contraction chunks of 128
NSQ = S_W // NC           # 4 sq chunks
NSKB = S_W // 128         # 16 self key blocks
NSPB = S_P // 128         # 8 cross key blocks
NFB = FF_SH // 128        # 4 ffn hidden blocks per core


def decoder_kernel(tc):
    nc = tc.nc

    # all inputs host-prepacked to [128, ...] partition-major contiguous bf16
    wembT = nc.dram_tensor("wembT", [128, NSQ * DCH * NC], BF16,
                           kind="ExternalInput").ap()
    pembT = nc.dram_tensor("pembT", [128, 2 * DCH * NC], BF16,
                           kind="ExternalInput").ap()
    wqmT = nc.dram_tensor("wqmT", [128, DCH * HD], BF16, kind="ExternalInput").ap()
    wkmT = nc.dram_tensor("wkmT", [128, DCH * HD], BF16, kind="ExternalInput").ap()
    wvmT = nc.dram_tensor("wvmT", [128, DCH * HD], BF16, kind="ExternalInput").ap()
    wqcT = nc.dram_tensor("wqcT", [128, DCH * HD], BF16, kind="ExternalInput").ap()
    wkcT = nc.dram_tensor("wkcT", [128, DCH * HD], BF16, kind="ExternalInput").ap()
    wvcT = nc.dram_tensor("wvcT", [128, DCH * HD], BF16, kind="ExternalInput").ap()
    w1T = nc.dram_tensor("w1T", [128, DCH * FF_SH], BF16, kind="ExternalInput").ap()
    w2T = nc.dram_tensor("w2T", [128, DCH * NFB * 128], BF16,
                         kind="ExternalInput").ap()
    outT = nc.dram_tensor("outT", [D_MODEL, S_W], BF16, kind="ExternalOutput").ap()

    rg = [list(range(N_CORES))]

    with (
        tc.tile_pool(name="const", bufs=1) as constp,
        tc.tile_pool(name="dram", bufs=1, space="DRAM") as dramp,
        tc.tile_pool(name="big", bufs=1) as bigp,
        tc.tile_pool(name="chunk", bufs=2) as chkp,
        tc.tile_pool(name="work", bufs=2) as workp,
        tc.tile_pool(name="ps_pp", bufs=2, space="PSUM") as ps_pp,
        tc.tile_pool(name="ps_s", bufs=2, space="PSUM") as ps_s,
        tc.tile_pool(name="ps_o", bufs=1, space="PSUM") as ps_o,
    ):
        # ---- constants (issued before any gpsimd DMA so the mask isn't
        # stuck behind prefetch in the gpsimd instruction queue) ----
        ident = constp.tile([128, 128], BF16, tag="ident")
        make_identity(nc, ident[:])
        ones_col = constp.tile([128, 1], BF16, tag="ones_col")
        nc.vector.memset(ones_col[:], 1.0)
        # extended causal mask: mask_ext[x, yy] = 1 iff yy - x >= 384.
        # view k (k=0..3): mask_ext[:, 384-128k : 896-128k] gives
        # [x, y] = 1 iff y - x >= 128k.
        mask_ext = constp.tile([128, 896], BF16, tag="mask_ext")
        nc.vector.memset(mask_ext[:], 1.0)
        nc.gpsimd.affine_select(
            out=mask_ext[:], in_=mask_ext[:],
            compare_op=mybir.AluOpType.is_ge,
            fill=0.0,
            base=-384,
            pattern=[[1, 896]],
            channel_multiplier=-1,
        )

        def mask_view(k):
            return mask_ext[:, 384 - 128 * k:896 - 128 * k]

        # ---- weight loads ----
        # All prefetch rides the gpsimd queue as FEW, BIG DMAs: the DMA
        # ring allows only ~3 outstanding per semaphore lane, so many
        # small DMAs stall the issuing engine's instruction queue on ring
        # credits (delaying everything queued behind them).
        def load_wT(dram_ap, tag, name, pieces=1):
            t = constp.tile([128, DCH * HD], BF16, tag=tag, name=name)
            q = DCH * HD // pieces
            for i in range(pieces):
                nc.gpsimd.dma_start(t[:, q * i:q * (i + 1)],
                                    dram_ap[:, q * i:q * (i + 1)])
            return t

        wq_sb = load_wT(wqmT, "wq", "wqm", pieces=2)
        wk_sb = load_wT(wkmT, "wk", "wkm")
        wv_sb = load_wT(wvmT, "wv", "wvm")

        def xcat_load(dram_ap, name, eng=None, tag="xcat", pieces=2):
            """prepacked [128, 8*512] DRAM block -> SBUF tile."""
            eng = eng or nc.sync
            t = chkp.tile([128, DCH * NC], BF16, tag=tag, name=name,
                          bufs=4 if tag == "xcat" else None)
            q = DCH * NC // pieces
            for i in range(pieces):
                eng.dma_start(t[:, q * i:q * (i + 1)],
                              dram_ap[:, q * i:q * (i + 1)])
            return [t[:, NC * dc:NC * (dc + 1)] for dc in range(DCH)]

        def dma_wemb(c):
            return xcat_load(
                wembT[:, DCH * NC * c:DCH * NC * (c + 1)], f"wemb_{c}")

        xc0 = dma_wemb(0)

        wqc_sb = load_wT(wqcT, "wq2", "wqc")
        wkc_sb = load_wT(wkcT, "wk2", "wkc")
        wvc_sb = load_wT(wvcT, "wv2", "wvc")

        # FFN weights, both resident in SBUF for the whole kernel
        w1_sb = constp.tile([128, DCH * FF_SH], BF16, tag="w1", name="w1")
        for i in range(2):
            q = DCH * FF_SH // 2
            nc.gpsimd.dma_start(w1_sb[:, q * i:q * (i + 1)],
                                w1T[:, q * i:q * (i + 1)])
        w2_sb = constp.tile([128, DCH * NFB * 128], BF16, tag="w2", name="w2")
        for i in range(2):
            q = DCH * NFB * 128 // 2
            nc.gpsimd.dma_start(w2_sb[:, q * i:q * (i + 1)],
                                w2T[:, q * i:q * (i + 1)])

        # pemb (both halves) prefetched on the gpsimd queue
        xp0 = xcat_load(pembT[:, 0:DCH * NC], "pemb_0", eng=nc.gpsimd,
                        tag="pcat")
        xp1 = xcat_load(pembT[:, DCH * NC:2 * DCH * NC], "pemb_1",
                        eng=nc.gpsimd, tag="pcat")

        # ---- self qkv projections, chunked over seq ----
        qT = bigp.tile([128, S_W], BF16, tag="qT", name="qT")
        kT = bigp.tile([128, S_W], BF16, tag="kT", name="kT")
        v65 = bigp.tile([128, NSKB * 130], BF16, tag="v65", name="v65")

        def proj_chunk(out_ap, w_sb, x_chunks):
            ps = ps_pp.tile([128, NC], F32, tag="pp", name="ps_pj")
            for dc in range(DCH):
                nc.tensor.matmul(
                    ps[:],
                    w_sb[:, HD * dc:HD * (dc + 1)],
                    x_chunks[dc][:],
                    start=(dc == 0),
                    stop=(dc == DCH - 1),
                )
            nc.vector.tensor_copy(out_ap, ps[:])

        def transp_block(v65_sb, vt_c, lb, b):
            ps = ps_pp.tile([128, 128], BF16, tag="pp", name="ps_tr")
            nc.tensor.transpose(ps[:], vt_c[:, 128 * lb:128 * (lb + 1)], ident[:])
            nc.vector.tensor_copy(v65_sb[:, 130 * b:130 * b + 64], ps[:, 0:64])
            nc.vector.tensor_copy(
                v65_sb[:, 130 * b + 65:130 * b + 129], ps[:, 64:128])
            nc.vector.tensor_copy(v65_sb[:, 130 * b + 64:130 * b + 65], ones_col[:])
            nc.vector.tensor_copy(
                v65_sb[:, 130 * b + 129:130 * b + 130], ones_col[:])

        # ---- attention chunk helper ----
        # Per j-step: both heads' scores go into one [128,1024] PSUM pair
        # (adjacent K=64 row-tiles, concurrent), ONE exp over both, then two
        # m=65 AV matmuls (ones-column -> softmax denominator in row 64).
        # `fillers` emits one unit of independent PE work after each j-step to
        # keep the PE dense through the ACT-bound exp chain.
        def attention_chunk(out_c, q_ap, k_sb, v65_sb, n_j, causal_c,
                            fillers=()):
            fill = iter(fillers)
            pso = [ps_o.tile([65, NC], F32, tag=f"o{h}", name=f"pso{h}")
                   for h in range(2)]
            for j in range(n_j):
                pss = ps_s.tile([128, 2 * NC], F32, tag="s", name="pss")
                for h in range(2):
                    nc.tensor.matmul(
                        pss[:, NC * h:NC * (h + 1)],
                        k_sb[64 * h:64 * (h + 1), 128 * j:128 * (j + 1)],
                        q_ap[64 * h:64 * (h + 1), :],
                        start=True, stop=True,
                        tile_position=(64 * h, 0),
                    )
                es = workp.tile([128, 2 * NC], BF16, tag="e", name="es")
                nc.scalar.activation(es[:], pss[:], AF.Exp, scale=0.125)
                if causal_c is not None and j >= 4 * causal_c:
                    for h in range(2):
                        nc.vector.tensor_mul(
                            es[:, NC * h:NC * (h + 1)],
                            es[:, NC * h:NC * (h + 1)],
                            mask_view(j - 4 * causal_c),
                        )
                for h in range(2):
                    nc.tensor.matmul(
                        pso[h][:],
                        v65_sb[:, 130 * j + 65 * h:130 * j + 65 * h + 65],
                        es[:, NC * h:NC * (h + 1)],
                        start=(j == 0),
                        stop=(j == n_j - 1),
                    )
                for th in (next(fill, None),):
                    if th is not None:
                        th()
            # copy the AV accumulators out of PSUM immediately: ps_o has
            # bufs=1, so the next chunk's first AV matmul WAR-waits on the
            # last read of pso — keep that read early, not at the end of
            # the reciprocal chain.
            av_sb = [workp.tile([64, NC], F32, tag=f"av{h}", name=f"av{h}")
                     for h in range(2)]
            lrow = [workp.tile([1, NC], F32, tag=f"lr{h}", name=f"lr{h}")
                    for h in range(2)]
            for h in range(2):
                nc.vector.tensor_copy(lrow[h][:], pso[h][64:65, :])
                nc.vector.tensor_copy(av_sb[h][:], pso[h][0:64, :])
            for th in fill:
                th()
            for h in range(2):
                rec = workp.tile([1, NC], F32, tag="rec", name="rec")
                nc.vector.reciprocal_approx_fast(rec[:], lrow[h][:])
                rec64 = workp.tile([64, NC], F32, tag="rec64", name="rec64")
                nc.gpsimd.partition_broadcast(rec64[:], rec[:])
                nc.vector.tensor_mul(
                    out_c[64 * h:64 * (h + 1), :], av_sb[h][:], rec64[:])

        # ---- work-unit emitters (used as attention fillers) ----
        kcT = bigp.tile([128, S_P], BF16, tag="kcT", name="kcT")
        vc65 = bigp.tile([128, NSPB * 130], BF16, tag="vc65", name="vc65")
        wd_c = {}
        cd_c = {}
        qc_t = {}

        def proj_q(c, xc):
            proj_chunk(qT[:, NC * c:NC * (c + 1)], wq_sb, xc)

        def proj_k(c, xc):
            proj_chunk(kT[:, NC * c:NC * (c + 1)], wk_sb, xc)

        def proj_v(c, xc):
            vtc = chkp.tile([128, NC], BF16, tag="vt", name=f"vT{c}", bufs=3)
            proj_chunk(vtc[:], wv_sb, xc)
            for lb in range(4):
                transp_block(v65, vtc, lb, 4 * c + lb)

        def proj_kc(sc, xc):
            proj_chunk(kcT[:, NC * sc:NC * (sc + 1)], wkc_sb, xc)

        def proj_vc(sc, xc):
            vtc = chkp.tile([128, NC], BF16, tag="vt", name=f"vcT{sc}", bufs=3)
            proj_chunk(vtc[:], wvc_sb, xc)
            for lb in range(4):
                transp_block(vc65, vtc, lb, 4 * sc + lb)

        def qc_proj(c):
            gath, off = wd_c[c]
            t = chkp.tile([128, DCH * NC], BF16, tag="wdcat",
                          name=f"word_{c}", bufs=4)
            for dc in range(DCH):
                nc.sync.dma_start(
                    t[:, NC * dc:NC * (dc + 1)],
                    gath[128 * dc:128 * (dc + 1), off:off + NC])
            xw = [t[:, NC * dc:NC * (dc + 1)] for dc in range(DCH)]
            qc = chkp.tile([128, NC], BF16, tag=f"qc{c % 2}", name=f"qcT{c}")
            proj_chunk(qc[:], wqc_sb, xw)
            qc_t[c] = qc

        def allgather(src_sb, name, width=NC):
            # bounce rides the gpsimd queue (the sync queue must stay clear
            # for wemb/gathered loads — an attention-gated bounce would
            # head-of-line block them)
            bounce = dramp.tile([128, width], BF16, name=f"bnc_{name}")
            gath = dramp.tile([N_CORES * 128, width], BF16, name=f"gd_{name}",
                              addr_space="Shared")
            nc.gpsimd.dma_start(bounce[:], src_sb[:])
            nc.gpsimd.collective_compute(
                "AllGather",
                mybir.AluOpType.bypass,
                replica_groups=rg,
                ins=[bounce[:].opt()],
                outs=[gath[:].opt()],
            )
            return gath

        ffn_state = {}

        def ffn_load(c):
            t = chkp.tile([128, DCH * NC], BF16, tag="xcat", name=f"cr_{c}",
                          bufs=4)
            for dc in range(DCH):
                nc.sync.dma_start(
                    t[:, NC * dc:NC * (dc + 1)],
                    cd_c[c][128 * dc:128 * (dc + 1), :])
            xc = [t[:, NC * dc:NC * (dc + 1)] for dc in range(DCH)]
            ffn_state[c] = (xc, [])

        def ffn1(c, fb):
            xc, hts = ffn_state[c]
            ps = ps_pp.tile([128, NC], F32, tag="pp", name="ps_f1")
            for dc in range(DCH):
                nc.tensor.matmul(
                    ps[:],
                    w1_sb[:, FF_SH * dc + 128 * fb:FF_SH * dc + 128 * (fb + 1)],
                    xc[dc][:],
                    start=(dc == 0),
                    stop=(dc == DCH - 1),
                )
            ht = chkp.tile([128, NC], BF16, tag=f"h{fb}", name=f"hT{fb}_{c}",
                           bufs=1)
            nc.vector.tensor_relu(ht[:], ps[:])
            hts.append(ht)

        def ffn2(c, ob):
            hts = ffn_state[c][1]
            ps = ps_pp.tile([128, NC], F32, tag="pp", name="ps_f2")
            for fc in range(NFB):
                nc.tensor.matmul(
                    ps[:],
                    w2_sb[:, 512 * ob + 128 * fc:512 * ob + 128 * (fc + 1)],
                    hts[fc][:],
                    start=(fc == 0),
                    stop=(fc == NFB - 1),
                )
            o_sb = workp.tile([128, NC], BF16, tag="o_sb", name="o_sb")
            nc.vector.tensor_copy(o_sb[:], ps[:])
            eng = nc.sync if ob % 2 else nc.gpsimd
            eng.dma_start(
                outT[128 * ob:128 * (ob + 1), NC * c:NC * (c + 1)], o_sb[:])

        def ffn_thunks(c):
            ts = [lambda c=c: ffn_load(c)]
            ts += [lambda c=c, fb=fb: ffn1(c, fb) for fb in range(NFB)]
            ts += [lambda c=c, ob=ob: ffn2(c, ob) for ob in range(DCH)]
            return ts

        # ---- the pipeline ----
        # AGs fire per 512-chunk, right after the chunk's attention output;
        # AG-dependent fillers are placed at least one full chunk later so
        # the in-order PE queue never head-of-line blocks on a collective.

        # filler schedules per self chunk (n_j = 4, 8, 12, 16).
        # Chunk c's attention carries chunk c+1's wemb DMA + projections as
        # fillers (PE work to hide the exp chain), plus the cross-side prep.
        # qc_proj(c) needs AG of self chunk c: schedule it at least one
        # full attention chunk after that AG fires, so the in-order PE
        # queue never head-of-line blocks on the collective.
        xq = {0: xc0}

        def next_proj(c):
            if c >= NSQ:
                return []
            ts = [lambda: xq.__setitem__(c, dma_wemb(c)) if c not in xq
                  else None]
            ts += [lambda: proj_q(c, xq[c]), lambda: proj_k(c, xq[c]),
                   lambda: proj_v(c, xq[c])]
            return ts

        def self_fillers(c):
            ts = next_proj(c + 1)
            if c == 2:
                ts += [lambda: proj_kc(0, xp0), lambda: proj_vc(0, xp0)]
            if c == 3:
                ts += [lambda: proj_kc(1, xp1), lambda: proj_vc(1, xp1),
                       lambda: qc_proj(0)]
            return ts

        # self chunks 0+1 share one paired AllGather: AG consumers wait on a
        # shared collective-completion counter, so qc_proj(0) effectively
        # waits for every AG issued before it anyway — pairing makes the
        # pair's data land in one (earlier-finishing) op.
        xq[1] = dma_wemb(1)
        proj_q(0, xq[0])
        proj_k(0, xq[0])
        proj_v(0, xq[0])
        self_pair = chkp.tile([128, 2 * NC], BF16, tag="oa01", name="selfP01")
        for c in range(NSQ):
            if c < 2:
                self_out = self_pair[:, NC * c:NC * (c + 1)]
            else:
                self_out = chkp.tile([128, NC], BF16, tag=f"oa{c % 2}",
                                     name=f"selfO{c}")[:]
            attention_chunk(self_out,
                            qT[:, NC * c:NC * (c + 1)], kT, v65,
                            4 * (c + 1), causal_c=c, fillers=self_fillers(c))
            if c == 1:
                g = allgather(self_pair[:], "w01", width=2 * NC)
                wd_c[0] = (g, 0)
                wd_c[1] = (g, NC)
            elif c >= 2:
                wd_c[c] = (allgather(self_out, f"w{c}"), 0)

        # cross chunks with qc/FFN fillers
        def cross_fillers(c):
            ts = []
            if c == 0:
                ts += [lambda: qc_proj(1)]
            if c == 1:
                ts += [lambda: qc_proj(2)]
            if c == 2:
                ts += [lambda: qc_proj(3)]
                ts += ffn_thunks(0)
            if c == 3:
                ts += ffn_thunks(1)
            return ts

        for c in range(NSQ):
            cross_out = chkp.tile([128, NC], BF16, tag=f"oa{c % 2}",
                                  name=f"crossO{c}")
            attention_chunk(cross_out[:],
                            qc_t[c][:], kcT, vc65, NSPB,
                            causal_c=None, fillers=cross_fillers(c))
            cd_c[c] = allgather(cross_out, f"c{c}")

        for th in ffn_thunks(NSQ - 2):
            th()
        for th in ffn_thunks(NSQ - 1):
            th()


_CACHED_NC = None


def _build():
    global _CACHED_NC
    if _CACHED_NC is None:
        nc = bacc.Bacc(
            "TRN2",
            target_bir_lowering=False,
            debug=False,
            num_devices=N_CORES,
        )
        with tile.TileContext(nc) as tc:
            decoder_kernel(tc)
        nc.compile()
        _CACHED_NC = nc
    return _CACHED_NC


def _pack_w(wT):
    """[1024, m] -> [128, 8*m]: d-chunk blocks side by side, partition-major."""
    m = wT.shape[1]
    return np.ascontiguousarray(
        wT.reshape(8, 128, m).transpose(1, 0, 2).reshape(128, 8 * m)
    ).astype(ml_dtypes.bfloat16)


def _pack_x(xT, nch):
    """[1024, nch*512] -> [128, nch * 8 * 512]: per seq-chunk c, the 8
    feature-blocks of that chunk's columns, contiguous."""
    return np.ascontiguousarray(
        xT.reshape(8, 128, nch, 512).transpose(1, 2, 0, 3)
        .reshape(128, nch * 8 * 512)).astype(ml_dtypes.bfloat16)


def make_in_maps(inputs):
    """Host-side prep: transposes + per-core weight slices + prepack."""
    f = np.ascontiguousarray
    wembT = _pack_x(np.asarray(inputs["wemb"], np.float32).T, NSQ)
    pembT = _pack_x(np.asarray(inputs["pemb"], np.float32).T, 2)
    in_maps = []
    for i in range(N_CORES):
        hsl = slice(HD * i, HD * (i + 1))
        fsl = slice(FF_SH * i, FF_SH * (i + 1))
        w2T = np.asarray(inputs["W2"], np.float32)[:, fsl].T  # [512, 1024]
        w2h = f(w2T.reshape(4, 128, 8, 128).transpose(1, 2, 0, 3)
                .reshape(128, 4096)).astype(ml_dtypes.bfloat16)
        in_maps.append({
            "wembT": wembT,
            "pembT": pembT,
            "wqmT": _pack_w(np.asarray(inputs["Wq_m"], np.float32)[hsl, :].T),
            "wkmT": _pack_w(np.asarray(inputs["Wk_m"], np.float32)[hsl, :].T),
            "wvmT": _pack_w(np.asarray(inputs["Wv_m"], np.float32)[hsl, :].T),
            "wqcT": _pack_w(np.asarray(inputs["Wq_c"], np.float32)[hsl, :].T),
            "wkcT": _pack_w(np.asarray(inputs["Wk_c"], np.float32)[hsl, :].T),
            "wvcT": _pack_w(np.asarray(inputs["Wv_c"], np.float32)[hsl, :].T),
            "w1T": _pack_w(np.asarray(inputs["W1"], np.float32)[fsl, :].T),
            "w2T": w2h,
        })
    return in_maps


def kernel(**inputs) -> np.ndarray:
    nc = _build()
    in_maps = make_in_maps(inputs)
    res = run_bass_kernel_spmd(nc, in_maps, core_ids=list(range(N_CORES)))
    acc = np.zeros((D_MODEL, S_W), dtype=np.float32)
    for i in range(N_CORES):
        acc += np.asarray(res.results[i]["outT"], np.float32)
    return np.ascontiguousarray(acc.T.astype(np.float32))


# revision 27
# speedup vs baseline: 1.1474x; 1.1474x over previous
"""Trainium2 Bass kernel for nn_Decoder2 (dense transformer decoder block).

Sharding (8 cores):
  - both attentions: head-sharded, 2 heads (=128 feature dims) per core
  - FFN: hidden dim column/row sharded, 512 hidden units per core; the 8
    partial outputs are summed on the host
  - wemb/pemb replicated; all activations kept transposed [feat, seq]

The kernel is a software pipeline over 4 sequence chunks of 512: each
chunk's self-attention output is AllGathered independently, so the
collectives and the cross-attention/FFN for chunk c overlap later
attention chunks. Same for the cross->FFN boundary.

All matmul operands are bf16 (PE moving-operand rate is higher than
fp32r and DMA bytes halve); PSUM accumulation stays fp32, softmax
denominator/reciprocal stay fp32. Softmax is computed without
max-subtraction (scores are O(+-6)); the denominator comes from a
ones-column folded into the AV matmul (lhsT = [v_head | ones], m=65).
Scores for the two heads are issued adjacently as K=64 row-tiles
(tile_position) so they run concurrently on the PE.

Bulk weight prefetch rides the GpSimd DMA queue so the first wemb
chunk (Sync queue) isn't stuck behind it; w2 stays SBUF-resident.
"""

import ml_dtypes
import numpy as np

import concourse.bass as bass
import concourse.bacc as bacc
import concourse.mybir as mybir
import concourse.tile as tile
from concourse.bass_utils import run_bass_kernel_spmd
from concourse.masks import make_identity

F32 = mybir.dt.float32
BF16 = mybir.dt.bfloat16
AF = mybir.ActivationFunctionType

N_CORES = 8
S_W, S_P = 2048, 1024
D_MODEL, NEW_DIM, H, D_FF = 1024, 1024, 16, 4096
HD = 128          # head-feature dims per core (2 heads x 64)
FF_SH = D_FF // N_CORES   # 512 hidden units per core
NC = 512          # free-dim chunk for matmuls
DCH = D_MODEL // 128      # 8 contraction chunks of 128
NSQ = S_W // NC           # 4 sq chunks
NSKB = S_W // 128         # 16 self key blocks
NSPB = S_P // 128         # 8 cross key blocks
NFB = FF_SH // 128        # 4 ffn hidden blocks per core


def decoder_kernel(tc):
    nc = tc.nc

    # all inputs host-prepacked to [128, ...] partition-major contiguous bf16
    wembT = nc.dram_tensor("wembT", [128, NSQ * DCH * NC], BF16,
                           kind="ExternalInput").ap()
    pembT = nc.dram_tensor("pembT", [128, 2 * DCH * NC], BF16,
                           kind="ExternalInput").ap()
    wqmT = nc.dram_tensor("wqmT", [128, DCH * HD], BF16, kind="ExternalInput").ap()
    wkmT = nc.dram_tensor("wkmT", [128, DCH * HD], BF16, kind="ExternalInput").ap()
    wvmT = nc.dram_tensor("wvmT", [128, DCH * HD], BF16, kind="ExternalInput").ap()
    wqcT = nc.dram_tensor("wqcT", [128, DCH * HD], BF16, kind="ExternalInput").ap()
    wkcT = nc.dram_tensor("wkcT", [128, DCH * HD], BF16, kind="ExternalInput").ap()
    wvcT = nc.dram_tensor("wvcT", [128, DCH * HD], BF16, kind="ExternalInput").ap()
    w1T = nc.dram_tensor("w1T", [128, DCH * FF_SH], BF16, kind="ExternalInput").ap()
    w2T = nc.dram_tensor("w2T", [128, DCH * NFB * 128], BF16,
                         kind="ExternalInput").ap()
    outT = nc.dram_tensor("outT", [D_MODEL, S_W], BF16, kind="ExternalOutput").ap()

    rg = [list(range(N_CORES))]

    with (
        tc.tile_pool(name="const", bufs=1) as constp,
        tc.tile_pool(name="dram", bufs=1, space="DRAM") as dramp,
        tc.tile_pool(name="big", bufs=1) as bigp,
        tc.tile_pool(name="chunk", bufs=2) as chkp,
        tc.tile_pool(name="work", bufs=2) as workp,
        tc.tile_pool(name="ps_pp", bufs=2, space="PSUM") as ps_pp,
        tc.tile_pool(name="ps_s", bufs=2, space="PSUM") as ps_s,
        tc.tile_pool(name="ps_o", bufs=1, space="PSUM") as ps_o,
    ):
        # ---- collective-stream warmup, the very first gpsimd work ----
        # The cross-core entry barrier of the FIRST collective completes
        # only once every core has triggered its first collective op, so
        # the whole AG chain is gated on the slowest core's first trigger.
        # A tiny AllGather triggered at ~10us absorbs that sync while the
        # real work hasn't produced anything yet.
        warm_sb = constp.tile([128, 8], BF16, tag="warm")
        nc.vector.memset(warm_sb[:], 0.0)
        warm_bnc = dramp.tile([128, 8], BF16, name="warm_bnc")
        warm_gd = dramp.tile([N_CORES * 128, 8], BF16, name="warm_gd",
                             addr_space="Shared")
        nc.gpsimd.dma_start(warm_bnc[:], warm_sb[:])
        nc.gpsimd.collective_compute(
            "AllGather",
            mybir.AluOpType.bypass,
            replica_groups=rg,
            ins=[warm_bnc[:].opt()],
            outs=[warm_gd[:].opt()],
        )

        # ---- constants (issued before any gpsimd DMA so the mask isn't
        # stuck behind prefetch in the gpsimd instruction queue) ----
        ident = constp.tile([128, 128], BF16, tag="ident")
        make_identity(nc, ident[:])
        ones_col = constp.tile([128, 1], BF16, tag="ones_col")
        nc.vector.memset(ones_col[:], 1.0)
        # extended causal mask: mask_ext[x, yy] = 1 iff yy - x >= 384.
        # view k (k=0..3): mask_ext[:, 384-128k : 896-128k] gives
        # [x, y] = 1 iff y - x >= 128k.
        mask_ext = constp.tile([128, 896], BF16, tag="mask_ext")
        nc.vector.memset(mask_ext[:], 1.0)
        nc.gpsimd.affine_select(
            out=mask_ext[:], in_=mask_ext[:],
            compare_op=mybir.AluOpType.is_ge,
            fill=0.0,
            base=-384,
            pattern=[[1, 896]],
            channel_multiplier=-1,
        )

        def mask_view(k):
            return mask_ext[:, 384 - 128 * k:896 - 128 * k]

        # ---- weight loads ----
        # All prefetch rides the gpsimd queue as FEW, BIG DMAs: the DMA
        # ring allows only ~3 outstanding per semaphore lane, so many
        # small DMAs stall the issuing engine's instruction queue on ring
        # credits (delaying everything queued behind them).
        def load_wT(dram_ap, tag, name, pieces=1):
            t = constp.tile([128, DCH * HD], BF16, tag=tag, name=name)
            q = DCH * HD // pieces
            for i in range(pieces):
                nc.gpsimd.dma_start(t[:, q * i:q * (i + 1)],
                                    dram_ap[:, q * i:q * (i + 1)])
            return t

        wq_sb = load_wT(wqmT, "wq", "wqm", pieces=2)
        wk_sb = load_wT(wkmT, "wk", "wkm")
        wv_sb = load_wT(wvmT, "wv", "wvm")

        def xcat_load(dram_ap, name, eng=None, tag="xcat", pieces=2):
            """prepacked [128, 8*512] DRAM block -> SBUF tile."""
            eng = eng or nc.sync
            t = chkp.tile([128, DCH * NC], BF16, tag=tag, name=name,
                          bufs=4 if tag == "xcat" else None)
            q = DCH * NC // pieces
            for i in range(pieces):
                eng.dma_start(t[:, q * i:q * (i + 1)],
                              dram_ap[:, q * i:q * (i + 1)])
            return [t[:, NC * dc:NC * (dc + 1)] for dc in range(DCH)]

        def dma_wemb(c):
            return xcat_load(
                wembT[:, DCH * NC * c:DCH * NC * (c + 1)], f"wemb_{c}")

        xc0 = dma_wemb(0)

        wqc_sb = load_wT(wqcT, "wq2", "wqc")
        wkc_sb = load_wT(wkcT, "wk2", "wkc")
        wvc_sb = load_wT(wvcT, "wv2", "wvc")

        # FFN weights, both resident in SBUF for the whole kernel
        w1_sb = constp.tile([128, DCH * FF_SH], BF16, tag="w1", name="w1")
        for i in range(2):
            q = DCH * FF_SH // 2
            nc.gpsimd.dma_start(w1_sb[:, q * i:q * (i + 1)],
                                w1T[:, q * i:q * (i + 1)])
        w2_sb = constp.tile([128, DCH * NFB * 128], BF16, tag="w2", name="w2")
        for i in range(2):
            q = DCH * NFB * 128 // 2
            nc.gpsimd.dma_start(w2_sb[:, q * i:q * (i + 1)],
                                w2T[:, q * i:q * (i + 1)])

        # pemb (both halves) prefetched on the gpsimd queue
        xp0 = xcat_load(pembT[:, 0:DCH * NC], "pemb_0", eng=nc.gpsimd,
                        tag="pcat")
        xp1 = xcat_load(pembT[:, DCH * NC:2 * DCH * NC], "pemb_1",
                        eng=nc.gpsimd, tag="pcat")

        # ---- self qkv projections, chunked over seq ----
        qT = bigp.tile([128, S_W], BF16, tag="qT", name="qT")
        kT = bigp.tile([128, S_W], BF16, tag="kT", name="kT")
        v65 = bigp.tile([128, NSKB * 130], BF16, tag="v65", name="v65")

        def proj_chunk(out_ap, w_sb, x_chunks):
            ps = ps_pp.tile([128, NC], F32, tag="pp", name="ps_pj")
            for dc in range(DCH):
                nc.tensor.matmul(
                    ps[:],
                    w_sb[:, HD * dc:HD * (dc + 1)],
                    x_chunks[dc][:],
                    start=(dc == 0),
                    stop=(dc == DCH - 1),
                )
            nc.vector.tensor_copy(out_ap, ps[:])

        def transp_block(v65_sb, vt_c, lb, b):
            ps = ps_pp.tile([128, 128], BF16, tag="pp", name="ps_tr")
            nc.tensor.transpose(ps[:], vt_c[:, 128 * lb:128 * (lb + 1)], ident[:])
            nc.vector.tensor_copy(v65_sb[:, 130 * b:130 * b + 64], ps[:, 0:64])
            nc.vector.tensor_copy(
                v65_sb[:, 130 * b + 65:130 * b + 129], ps[:, 64:128])
            nc.vector.tensor_copy(v65_sb[:, 130 * b + 64:130 * b + 65], ones_col[:])
            nc.vector.tensor_copy(
                v65_sb[:, 130 * b + 129:130 * b + 130], ones_col[:])

        # ---- attention chunk helper ----
        # Per j-step: both heads' scores go into one [128,1024] PSUM pair
        # (adjacent K=64 row-tiles, concurrent), ONE exp over both, then two
        # m=65 AV matmuls (ones-column -> softmax denominator in row 64).
        # `fillers` emits one unit of independent PE work after each j-step to
        # keep the PE dense through the ACT-bound exp chain.
        def attention_chunk(out_c, q_ap, k_sb, v65_sb, n_j, causal_c,
                            fillers=()):
            fill = iter(fillers)
            pso = [ps_o.tile([65, NC], F32, tag=f"o{h}", name=f"pso{h}")
                   for h in range(2)]
            for j in range(n_j):
                pss = ps_s.tile([128, 2 * NC], F32, tag="s", name="pss")
                for h in range(2):
                    nc.tensor.matmul(
                        pss[:, NC * h:NC * (h + 1)],
                        k_sb[64 * h:64 * (h + 1), 128 * j:128 * (j + 1)],
                        q_ap[64 * h:64 * (h + 1), :],
                        start=True, stop=True,
                        tile_position=(64 * h, 0),
                    )
                es = workp.tile([128, 2 * NC], BF16, tag="e", name="es")
                nc.scalar.activation(es[:], pss[:], AF.Exp, scale=0.125)
                if causal_c is not None and j >= 4 * causal_c:
                    for h in range(2):
                        nc.vector.tensor_mul(
                            es[:, NC * h:NC * (h + 1)],
                            es[:, NC * h:NC * (h + 1)],
                            mask_view(j - 4 * causal_c),
                        )
                for h in range(2):
                    nc.tensor.matmul(
                        pso[h][:],
                        v65_sb[:, 130 * j + 65 * h:130 * j + 65 * h + 65],
                        es[:, NC * h:NC * (h + 1)],
                        start=(j == 0),
                        stop=(j == n_j - 1),
                    )
                for th in (next(fill, None),):
                    if th is not None:
                        th()
            # copy the AV accumulators out of PSUM immediately: ps_o has
            # bufs=1, so the next chunk's first AV matmul WAR-waits on the
            # last read of pso — keep that read early, not at the end of
            # the reciprocal chain.
            av_sb = [workp.tile([64, NC], F32, tag=f"av{h}", name=f"av{h}")
                     for h in range(2)]
            lrow = [workp.tile([1, NC], F32, tag=f"lr{h}", name=f"lr{h}")
                    for h in range(2)]
            for h in range(2):
                nc.vector.tensor_copy(lrow[h][:], pso[h][64:65, :])
                nc.vector.tensor_copy(av_sb[h][:], pso[h][0:64, :])
            for th in fill:
                th()
            for h in range(2):
                rec = workp.tile([1, NC], F32, tag="rec", name="rec")
                nc.vector.reciprocal_approx_fast(rec[:], lrow[h][:])
                rec64 = workp.tile([64, NC], F32, tag="rec64", name="rec64")
                nc.gpsimd.partition_broadcast(rec64[:], rec[:])
                nc.vector.tensor_mul(
                    out_c[64 * h:64 * (h + 1), :], av_sb[h][:], rec64[:])

        # ---- work-unit emitters (used as attention fillers) ----
        kcT = bigp.tile([128, S_P], BF16, tag="kcT", name="kcT")
        vc65 = bigp.tile([128, NSPB * 130], BF16, tag="vc65", name="vc65")
        wd_c = {}
        cd_c = {}
        qc_t = {}

        def proj_q(c, xc):
            proj_chunk(qT[:, NC * c:NC * (c + 1)], wq_sb, xc)

        def proj_k(c, xc):
            proj_chunk(kT[:, NC * c:NC * (c + 1)], wk_sb, xc)

        def proj_v(c, xc):
            vtc = chkp.tile([128, NC], BF16, tag="vt", name=f"vT{c}", bufs=3)
            proj_chunk(vtc[:], wv_sb, xc)
            for lb in range(4):
                transp_block(v65, vtc, lb, 4 * c + lb)

        def proj_kc(sc, xc):
            proj_chunk(kcT[:, NC * sc:NC * (sc + 1)], wkc_sb, xc)

        def proj_vc(sc, xc):
            vtc = chkp.tile([128, NC], BF16, tag="vt", name=f"vcT{sc}", bufs=3)
            proj_chunk(vtc[:], wvc_sb, xc)
            for lb in range(4):
                transp_block(vc65, vtc, lb, 4 * sc + lb)

        def qc_proj(c):
            gath, off = wd_c[c]
            t = chkp.tile([128, DCH * NC], BF16, tag="wdcat",
                          name=f"word_{c}", bufs=4)
            for dc in range(DCH):
                nc.sync.dma_start(
                    t[:, NC * dc:NC * (dc + 1)],
                    gath[128 * dc:128 * (dc + 1), off:off + NC])
            xw = [t[:, NC * dc:NC * (dc + 1)] for dc in range(DCH)]
            qc = chkp.tile([128, NC], BF16, tag=f"qc{c % 2}", name=f"qcT{c}")
            proj_chunk(qc[:], wqc_sb, xw)
            qc_t[c] = qc

        def allgather(src_sb, name, width=NC):
            # bounce rides the gpsimd queue (the sync queue must stay clear
            # for wemb/gathered loads — an attention-gated bounce would
            # head-of-line block them)
            bounce = dramp.tile([128, width], BF16, name=f"bnc_{name}")
            gath = dramp.tile([N_CORES * 128, width], BF16, name=f"gd_{name}",
                              addr_space="Shared")
            nc.gpsimd.dma_start(bounce[:], src_sb[:])
            nc.gpsimd.collective_compute(
                "AllGather",
                mybir.AluOpType.bypass,
                replica_groups=rg,
                ins=[bounce[:].opt()],
                outs=[gath[:].opt()],
            )
            return gath

        ffn_state = {}

        def ffn_load(c):
            t = chkp.tile([128, DCH * NC], BF16, tag="xcat", name=f"cr_{c}",
                          bufs=4)
            for dc in range(DCH):
                nc.sync.dma_start(
                    t[:, NC * dc:NC * (dc + 1)],
                    cd_c[c][128 * dc:128 * (dc + 1), :])
            xc = [t[:, NC * dc:NC * (dc + 1)] for dc in range(DCH)]
            ffn_state[c] = (xc, [])

        def ffn1(c, fb):
            xc, hts = ffn_state[c]
            ps = ps_pp.tile([128, NC], F32, tag="pp", name="ps_f1")
            for dc in range(DCH):
                nc.tensor.matmul(
                    ps[:],
                    w1_sb[:, FF_SH * dc + 128 * fb:FF_SH * dc + 128 * (fb + 1)],
                    xc[dc][:],
                    start=(dc == 0),
                    stop=(dc == DCH - 1),
                )
            ht = chkp.tile([128, NC], BF16, tag=f"h{fb}", name=f"hT{fb}_{c}",
                           bufs=1)
            nc.vector.tensor_relu(ht[:], ps[:])
            hts.append(ht)

        def ffn2(c, ob):
            hts = ffn_state[c][1]
            ps = ps_pp.tile([128, NC], F32, tag="pp", name="ps_f2")
            for fc in range(NFB):
                nc.tensor.matmul(
                    ps[:],
                    w2_sb[:, 512 * ob + 128 * fc:512 * ob + 128 * (fc + 1)],
                    hts[fc][:],
                    start=(fc == 0),
                    stop=(fc == NFB - 1),
                )
            o_sb = workp.tile([128, NC], BF16, tag="o_sb", name="o_sb")
            nc.vector.tensor_copy(o_sb[:], ps[:])
            eng = nc.sync if ob % 2 else nc.gpsimd
            eng.dma_start(
                outT[128 * ob:128 * (ob + 1), NC * c:NC * (c + 1)], o_sb[:])

        def ffn_thunks(c):
            ts = [lambda c=c: ffn_load(c)]
            ts += [lambda c=c, fb=fb: ffn1(c, fb) for fb in range(NFB)]
            ts += [lambda c=c, ob=ob: ffn2(c, ob) for ob in range(DCH)]
            return ts

        # ---- the pipeline ----
        # AGs fire per 512-chunk, right after the chunk's attention output;
        # AG-dependent fillers are placed at least one full chunk later so
        # the in-order PE queue never head-of-line blocks on a collective.

        # filler schedules per self chunk (n_j = 4, 8, 12, 16).
        # Chunk c's attention carries chunk c+1's wemb DMA + projections as
        # fillers (PE work to hide the exp chain), plus the cross-side prep.
        # qc_proj(c) needs AG of self chunk c: schedule it at least one
        # full attention chunk after that AG fires, so the in-order PE
        # queue never head-of-line blocks on the collective.
        xq = {0: xc0}

        def next_proj(c):
            if c >= NSQ:
                return []
            ts = [lambda: xq.__setitem__(c, dma_wemb(c)) if c not in xq
                  else None]
            ts += [lambda: proj_q(c, xq[c]), lambda: proj_k(c, xq[c]),
                   lambda: proj_v(c, xq[c])]
            return ts

        def self_fillers(c):
            ts = next_proj(c + 1)
            if c == 2:
                ts += [lambda: proj_kc(0, xp0), lambda: proj_vc(0, xp0)]
            if c == 3:
                ts += [lambda: proj_kc(1, xp1), lambda: proj_vc(1, xp1),
                       lambda: qc_proj(0)]
            return ts

        xq[1] = dma_wemb(1)
        proj_q(0, xq[0])
        proj_k(0, xq[0])
        proj_v(0, xq[0])
        for c in range(NSQ):
            self_out = chkp.tile([128, NC], BF16, tag=f"oa{c % 2}",
                                 name=f"selfO{c}")
            attention_chunk(self_out[:],
                            qT[:, NC * c:NC * (c + 1)], kT, v65,
                            4 * (c + 1), causal_c=c, fillers=self_fillers(c))
            wd_c[c] = (allgather(self_out, f"w{c}"), 0)

        # cross chunks with qc/FFN fillers
        def cross_fillers(c):
            ts = []
            if c == 0:
                ts += [lambda: qc_proj(1)]
            if c == 1:
                ts += [lambda: qc_proj(2)]
            if c == 2:
                ts += [lambda: qc_proj(3)]
                ts += ffn_thunks(0)
            if c == 3:
                ts += ffn_thunks(1)
            return ts

        for c in range(NSQ):
            cross_out = chkp.tile([128, NC], BF16, tag=f"oa{c % 2}",
                                  name=f"crossO{c}")
            attention_chunk(cross_out[:],
                            qc_t[c][:], kcT, vc65, NSPB,
                            causal_c=None, fillers=cross_fillers(c))
            cd_c[c] = allgather(cross_out, f"c{c}")

        for th in ffn_thunks(NSQ - 2):
            th()
        for th in ffn_thunks(NSQ - 1):
            th()


_CACHED_NC = None


def _build():
    global _CACHED_NC
    if _CACHED_NC is None:
        nc = bacc.Bacc(
            "TRN2",
            target_bir_lowering=False,
            debug=False,
            num_devices=N_CORES,
        )
        with tile.TileContext(nc) as tc:
            decoder_kernel(tc)
        nc.compile()
        _CACHED_NC = nc
    return _CACHED_NC


def _pack_w(wT):
    """[1024, m] -> [128, 8*m]: d-chunk blocks side by side, partition-major."""
    m = wT.shape[1]
    return np.ascontiguousarray(
        wT.reshape(8, 128, m).transpose(1, 0, 2).reshape(128, 8 * m)
    ).astype(ml_dtypes.bfloat16)


def _pack_x(xT, nch):
    """[1024, nch*512] -> [128, nch * 8 * 512]: per seq-chunk c, the 8
    feature-blocks of that chunk's columns, contiguous."""
    return np.ascontiguousarray(
        xT.reshape(8, 128, nch, 512).transpose(1, 2, 0, 3)
        .reshape(128, nch * 8 * 512)).astype(ml_dtypes.bfloat16)


def make_in_maps(inputs):
    """Host-side prep: transposes + per-core weight slices + prepack."""
    f = np.ascontiguousarray
    wembT = _pack_x(np.asarray(inputs["wemb"], np.float32).T, NSQ)
    pembT = _pack_x(np.asarray(inputs["pemb"], np.float32).T, 2)
    in_maps = []
    for i in range(N_CORES):
        hsl = slice(HD * i, HD * (i + 1))
        fsl = slice(FF_SH * i, FF_SH * (i + 1))
        w2T = np.asarray(inputs["W2"], np.float32)[:, fsl].T  # [512, 1024]
        w2h = f(w2T.reshape(4, 128, 8, 128).transpose(1, 2, 0, 3)
                .reshape(128, 4096)).astype(ml_dtypes.bfloat16)
        in_maps.append({
            "wembT": wembT,
            "pembT": pembT,
            "wqmT": _pack_w(np.asarray(inputs["Wq_m"], np.float32)[hsl, :].T),
            "wkmT": _pack_w(np.asarray(inputs["Wk_m"], np.float32)[hsl, :].T),
            "wvmT": _pack_w(np.asarray(inputs["Wv_m"], np.float32)[hsl, :].T),
            "wqcT": _pack_w(np.asarray(inputs["Wq_c"], np.float32)[hsl, :].T),
            "wkcT": _pack_w(np.asarray(inputs["Wk_c"], np.float32)[hsl, :].T),
            "wvcT": _pack_w(np.asarray(inputs["Wv_c"], np.float32)[hsl, :].T),
            "w1T": _pack_w(np.asarray(inputs["W1"], np.float32)[fsl, :].T),
            "w2T": w2h,
        })
    return in_maps


def kernel(**inputs) -> np.ndarray:
    nc = _build()
    in_maps = make_in_maps(inputs)
    res = run_bass_kernel_spmd(nc, in_maps, core_ids=list(range(N_CORES)))
    acc = np.zeros((D_MODEL, S_W), dtype=np.float32)
    for i in range(N_CORES):
        acc += np.asarray(res.results[i]["outT"], np.float32)
    return np.ascontiguousarray(acc.T.astype(np.float32))


# revision 28
# speedup vs baseline: 1.1907x; 1.0378x over previous
"""Trainium2 Bass kernel for nn_Decoder2 (dense transformer decoder block).

Sharding (8 cores):
  - both attentions: head-sharded, 2 heads (=128 feature dims) per core
  - FFN: hidden dim column/row sharded, 512 hidden units per core; the 8
    partial outputs are summed on the host
  - wemb/pemb replicated; all activations kept transposed [feat, seq]

The kernel is a software pipeline over 4 sequence chunks of 512: each
chunk's self-attention output is AllGathered independently, so the
collectives and the cross-attention/FFN for chunk c overlap later
attention chunks. Same for the cross->FFN boundary.

All matmul operands are bf16 (PE moving-operand rate is higher than
fp32r and DMA bytes halve); PSUM accumulation stays fp32, softmax
denominator/reciprocal stay fp32. Softmax is computed without
max-subtraction (scores are O(+-6)); the denominator comes from a
ones-column folded into the AV matmul (lhsT = [v_head | ones], m=65).
Scores for the two heads are issued adjacently as K=64 row-tiles
(tile_position) so they run concurrently on the PE.

Bulk weight prefetch rides the GpSimd DMA queue so the first wemb
chunk (Sync queue) isn't stuck behind it; w2 stays SBUF-resident.
"""

import ml_dtypes
import numpy as np

import concourse.bass as bass
import concourse.bacc as bacc
import concourse.mybir as mybir
import concourse.tile as tile
from concourse.bass_utils import run_bass_kernel_spmd
from concourse.masks import make_identity

F32 = mybir.dt.float32
BF16 = mybir.dt.bfloat16
AF = mybir.ActivationFunctionType

N_CORES = 8
S_W, S_P = 2048, 1024
D_MODEL, NEW_DIM, H, D_FF = 1024, 1024, 16, 4096
HD = 128          # head-feature dims per core (2 heads x 64)
FF_SH = D_FF // N_CORES   # 512 hidden units per core
NC = 512          # free-dim chunk for matmuls
DCH = D_MODEL // 128      # 8 contraction chunks of 128
NSQ = S_W // NC           # 4 sq chunks
NSKB = S_W // 128         # 16 self key blocks
NSPB = S_P // 128         # 8 cross key blocks
NFB = FF_SH // 128        # 4 ffn hidden blocks per core


def decoder_kernel(tc):
    nc = tc.nc

    # all inputs host-prepacked to [128, ...] partition-major contiguous bf16
    wembT = nc.dram_tensor("wembT", [128, NSQ * DCH * NC], BF16,
                           kind="ExternalInput").ap()
    pembT = nc.dram_tensor("pembT", [128, 2 * DCH * NC], BF16,
                           kind="ExternalInput").ap()
    wqmT = nc.dram_tensor("wqmT", [128, DCH * HD], BF16, kind="ExternalInput").ap()
    wkmT = nc.dram_tensor("wkmT", [128, DCH * HD], BF16, kind="ExternalInput").ap()
    wvmT = nc.dram_tensor("wvmT", [128, DCH * HD], BF16, kind="ExternalInput").ap()
    wqcT = nc.dram_tensor("wqcT", [128, DCH * HD], BF16, kind="ExternalInput").ap()
    wkcT = nc.dram_tensor("wkcT", [128, DCH * HD], BF16, kind="ExternalInput").ap()
    wvcT = nc.dram_tensor("wvcT", [128, DCH * HD], BF16, kind="ExternalInput").ap()
    w1T = nc.dram_tensor("w1T", [128, DCH * FF_SH], BF16, kind="ExternalInput").ap()
    w2T = nc.dram_tensor("w2T", [128, DCH * NFB * 128], BF16,
                         kind="ExternalInput").ap()
    outT = nc.dram_tensor("outT", [D_MODEL, S_W], BF16, kind="ExternalOutput").ap()

    rg = [list(range(N_CORES))]

    with (
        tc.tile_pool(name="const", bufs=1) as constp,
        tc.tile_pool(name="dram", bufs=1, space="DRAM") as dramp,
        tc.tile_pool(name="big", bufs=1) as bigp,
        tc.tile_pool(name="chunk", bufs=2) as chkp,
        tc.tile_pool(name="work", bufs=2) as workp,
        tc.tile_pool(name="ps_pp", bufs=2, space="PSUM") as ps_pp,
        tc.tile_pool(name="ps_s", bufs=2, space="PSUM") as ps_s,
        tc.tile_pool(name="ps_o", bufs=1, space="PSUM") as ps_o,
    ):
        # ---- constants (issued before any gpsimd DMA so the mask isn't
        # stuck behind prefetch in the gpsimd instruction queue) ----
        ident = constp.tile([128, 128], BF16, tag="ident")
        make_identity(nc, ident[:])
        ones_col = constp.tile([128, 1], BF16, tag="ones_col")
        nc.vector.memset(ones_col[:], 1.0)
        # extended causal mask: mask_ext[x, yy] = 1 iff yy - x >= 384.
        # view k (k=0..3): mask_ext[:, 384-128k : 896-128k] gives
        # [x, y] = 1 iff y - x >= 128k.
        mask_ext = constp.tile([128, 896], BF16, tag="mask_ext")
        nc.vector.memset(mask_ext[:], 1.0)
        nc.gpsimd.affine_select(
            out=mask_ext[:], in_=mask_ext[:],
            compare_op=mybir.AluOpType.is_ge,
            fill=0.0,
            base=-384,
            pattern=[[1, 896]],
            channel_multiplier=-1,
        )

        def mask_view(k):
            return mask_ext[:, 384 - 128 * k:896 - 128 * k]

        # ---- weight loads ----
        # All prefetch rides the gpsimd queue as FEW, BIG DMAs: the DMA
        # ring allows only ~3 outstanding per semaphore lane, so many
        # small DMAs stall the issuing engine's instruction queue on ring
        # credits (delaying everything queued behind them).
        def load_wT(dram_ap, tag, name, pieces=1):
            t = constp.tile([128, DCH * HD], BF16, tag=tag, name=name)
            q = DCH * HD // pieces
            for i in range(pieces):
                nc.gpsimd.dma_start(t[:, q * i:q * (i + 1)],
                                    dram_ap[:, q * i:q * (i + 1)])
            return t

        wq_sb = load_wT(wqmT, "wq", "wqm", pieces=2)
        wk_sb = load_wT(wkmT, "wk", "wkm")
        wv_sb = load_wT(wvmT, "wv", "wvm")

        def xcat_load(dram_ap, name, eng=None, tag="xcat", pieces=2):
            """prepacked [128, 8*512] DRAM block -> SBUF tile."""
            eng = eng or nc.sync
            t = chkp.tile([128, DCH * NC], BF16, tag=tag, name=name,
                          bufs=4 if tag == "xcat" else None)
            q = DCH * NC // pieces
            for i in range(pieces):
                eng.dma_start(t[:, q * i:q * (i + 1)],
                              dram_ap[:, q * i:q * (i + 1)])
            return [t[:, NC * dc:NC * (dc + 1)] for dc in range(DCH)]

        def dma_wemb(c):
            return xcat_load(
                wembT[:, DCH * NC * c:DCH * NC * (c + 1)], f"wemb_{c}")

        xc0 = dma_wemb(0)

        wqc_sb = load_wT(wqcT, "wq2", "wqc")
        wkc_sb = load_wT(wkcT, "wk2", "wkc")
        wvc_sb = load_wT(wvcT, "wv2", "wvc")

        # FFN weights, both resident in SBUF for the whole kernel
        w1_sb = constp.tile([128, DCH * FF_SH], BF16, tag="w1", name="w1")
        for i in range(2):
            q = DCH * FF_SH // 2
            nc.gpsimd.dma_start(w1_sb[:, q * i:q * (i + 1)],
                                w1T[:, q * i:q * (i + 1)])
        w2_sb = constp.tile([128, DCH * NFB * 128], BF16, tag="w2", name="w2")
        for i in range(2):
            q = DCH * NFB * 128 // 2
            nc.gpsimd.dma_start(w2_sb[:, q * i:q * (i + 1)],
                                w2T[:, q * i:q * (i + 1)])

        # pemb (both halves) prefetched on the gpsimd queue
        xp0 = xcat_load(pembT[:, 0:DCH * NC], "pemb_0", eng=nc.gpsimd,
                        tag="pcat")
        xp1 = xcat_load(pembT[:, DCH * NC:2 * DCH * NC], "pemb_1",
                        eng=nc.gpsimd, tag="pcat")

        # ---- self qkv projections, chunked over seq ----
        qT = bigp.tile([128, S_W], BF16, tag="qT", name="qT")
        kT = bigp.tile([128, S_W], BF16, tag="kT", name="kT")
        v65 = bigp.tile([128, NSKB * 130], BF16, tag="v65", name="v65")

        def proj_chunk(out_ap, w_sb, x_chunks):
            ps = ps_pp.tile([128, NC], F32, tag="pp", name="ps_pj")
            for dc in range(DCH):
                nc.tensor.matmul(
                    ps[:],
                    w_sb[:, HD * dc:HD * (dc + 1)],
                    x_chunks[dc][:],
                    start=(dc == 0),
                    stop=(dc == DCH - 1),
                )
            nc.vector.tensor_copy(out_ap, ps[:])

        def transp_block(v65_sb, vt_c, lb, b):
            ps = ps_pp.tile([128, 128], BF16, tag="pp", name="ps_tr")
            nc.tensor.transpose(ps[:], vt_c[:, 128 * lb:128 * (lb + 1)], ident[:])
            nc.vector.tensor_copy(v65_sb[:, 130 * b:130 * b + 64], ps[:, 0:64])
            nc.vector.tensor_copy(
                v65_sb[:, 130 * b + 65:130 * b + 129], ps[:, 64:128])
            nc.vector.tensor_copy(v65_sb[:, 130 * b + 64:130 * b + 65], ones_col[:])
            nc.vector.tensor_copy(
                v65_sb[:, 130 * b + 129:130 * b + 130], ones_col[:])

        # ---- attention chunk helper ----
        # Per j-step: both heads' scores go into one [128,1024] PSUM pair
        # (adjacent K=64 row-tiles, concurrent), ONE exp over both, then two
        # m=65 AV matmuls (ones-column -> softmax denominator in row 64).
        # `fillers` emits one unit of independent PE work after each j-step to
        # keep the PE dense through the ACT-bound exp chain.
        def attention_chunk(out_c, q_ap, k_sb, v65_sb, n_j, causal_c,
                            fillers=()):
            fill = iter(fillers)
            pso = [ps_o.tile([65, NC], F32, tag=f"o{h}", name=f"pso{h}")
                   for h in range(2)]
            for j in range(n_j):
                pss = ps_s.tile([128, 2 * NC], F32, tag="s", name="pss")
                for h in range(2):
                    nc.tensor.matmul(
                        pss[:, NC * h:NC * (h + 1)],
                        k_sb[64 * h:64 * (h + 1), 128 * j:128 * (j + 1)],
                        q_ap[64 * h:64 * (h + 1), :],
                        start=True, stop=True,
                        tile_position=(64 * h, 0),
                    )
                es = workp.tile([128, 2 * NC], BF16, tag="e", name="es")
                nc.scalar.activation(es[:], pss[:], AF.Exp, scale=0.125)
                if causal_c is not None and j >= 4 * causal_c:
                    for h in range(2):
                        nc.vector.tensor_mul(
                            es[:, NC * h:NC * (h + 1)],
                            es[:, NC * h:NC * (h + 1)],
                            mask_view(j - 4 * causal_c),
                        )
                for h in range(2):
                    nc.tensor.matmul(
                        pso[h][:],
                        v65_sb[:, 130 * j + 65 * h:130 * j + 65 * h + 65],
                        es[:, NC * h:NC * (h + 1)],
                        start=(j == 0),
                        stop=(j == n_j - 1),
                    )
                for th in (next(fill, None),):
                    if th is not None:
                        th()
            # copy the AV accumulators out of PSUM immediately: ps_o has
            # bufs=1, so the next chunk's first AV matmul WAR-waits on the
            # last read of pso — keep that read early, not at the end of
            # the reciprocal chain.
            av_sb = [workp.tile([64, NC], F32, tag=f"av{h}", name=f"av{h}")
                     for h in range(2)]
            lrow = [workp.tile([1, NC], F32, tag=f"lr{h}", name=f"lr{h}")
                    for h in range(2)]
            for h in range(2):
                nc.vector.tensor_copy(lrow[h][:], pso[h][64:65, :])
                nc.vector.tensor_copy(av_sb[h][:], pso[h][0:64, :])
            for th in fill:
                th()
            for h in range(2):
                rec = workp.tile([1, NC], F32, tag="rec", name="rec")
                nc.vector.reciprocal_approx_fast(rec[:], lrow[h][:])
                rec64 = workp.tile([64, NC], F32, tag="rec64", name="rec64")
                nc.gpsimd.partition_broadcast(rec64[:], rec[:])
                nc.vector.tensor_mul(
                    out_c[64 * h:64 * (h + 1), :], av_sb[h][:], rec64[:])

        # ---- work-unit emitters (used as attention fillers) ----
        kcT = bigp.tile([128, S_P], BF16, tag="kcT", name="kcT")
        vc65 = bigp.tile([128, NSPB * 130], BF16, tag="vc65", name="vc65")
        wd_c = {}
        cd_c = {}
        qc_t = {}

        def proj_q(c, xc):
            proj_chunk(qT[:, NC * c:NC * (c + 1)], wq_sb, xc)

        def proj_k(c, xc):
            proj_chunk(kT[:, NC * c:NC * (c + 1)], wk_sb, xc)

        def proj_v(c, xc):
            vtc = chkp.tile([128, NC], BF16, tag="vt", name=f"vT{c}", bufs=3)
            proj_chunk(vtc[:], wv_sb, xc)
            for lb in range(4):
                transp_block(v65, vtc, lb, 4 * c + lb)

        def proj_kc(sc, xc):
            proj_chunk(kcT[:, NC * sc:NC * (sc + 1)], wkc_sb, xc)

        def proj_vc(sc, xc):
            vtc = chkp.tile([128, NC], BF16, tag="vt", name=f"vcT{sc}", bufs=3)
            proj_chunk(vtc[:], wvc_sb, xc)
            for lb in range(4):
                transp_block(vc65, vtc, lb, 4 * sc + lb)

        def qc_proj(c):
            gath, off = wd_c[c]
            t = chkp.tile([128, DCH * NC], BF16, tag="wdcat",
                          name=f"word_{c}", bufs=4)
            for dc in range(DCH):
                nc.sync.dma_start(
                    t[:, NC * dc:NC * (dc + 1)],
                    gath[128 * dc:128 * (dc + 1), off:off + NC])
            xw = [t[:, NC * dc:NC * (dc + 1)] for dc in range(DCH)]
            qc = chkp.tile([128, NC], BF16, tag=f"qc{c % 2}", name=f"qcT{c}")
            proj_chunk(qc[:], wqc_sb, xw)
            qc_t[c] = qc

        def allgather(src_sb, name, width=NC):
            # bounce rides the gpsimd queue (the sync queue must stay clear
            # for wemb/gathered loads — an attention-gated bounce would
            # head-of-line block them)
            bounce = dramp.tile([128, width], BF16, name=f"bnc_{name}")
            gath = dramp.tile([N_CORES * 128, width], BF16, name=f"gd_{name}",
                              addr_space="Shared")
            nc.gpsimd.dma_start(bounce[:], src_sb[:])
            nc.gpsimd.collective_compute(
                "AllGather",
                mybir.AluOpType.bypass,
                replica_groups=rg,
                ins=[bounce[:].opt()],
                outs=[gath[:].opt()],
            )
            return gath

        ffn_state = {}

        def ffn_load(c):
            t = chkp.tile([128, DCH * NC], BF16, tag="xcat", name=f"cr_{c}",
                          bufs=4)
            for dc in range(DCH):
                nc.sync.dma_start(
                    t[:, NC * dc:NC * (dc + 1)],
                    cd_c[c][128 * dc:128 * (dc + 1), :])
            xc = [t[:, NC * dc:NC * (dc + 1)] for dc in range(DCH)]
            ffn_state[c] = (xc, [])

        def ffn1(c, fb):
            xc, hts = ffn_state[c]
            ps = ps_pp.tile([128, NC], F32, tag="pp", name="ps_f1")
            for dc in range(DCH):
                nc.tensor.matmul(
                    ps[:],
                    w1_sb[:, FF_SH * dc + 128 * fb:FF_SH * dc + 128 * (fb + 1)],
                    xc[dc][:],
                    start=(dc == 0),
                    stop=(dc == DCH - 1),
                )
            ht = chkp.tile([128, NC], BF16, tag=f"h{fb}", name=f"hT{fb}_{c}",
                           bufs=1)
            nc.vector.tensor_relu(ht[:], ps[:])
            hts.append(ht)

        def ffn2(c, ob):
            hts = ffn_state[c][1]
            ps = ps_pp.tile([128, NC], F32, tag="pp", name="ps_f2")
            for fc in range(NFB):
                nc.tensor.matmul(
                    ps[:],
                    w2_sb[:, 512 * ob + 128 * fc:512 * ob + 128 * (fc + 1)],
                    hts[fc][:],
                    start=(fc == 0),
                    stop=(fc == NFB - 1),
                )
            o_sb = workp.tile([128, NC], BF16, tag="o_sb", name="o_sb")
            nc.vector.tensor_copy(o_sb[:], ps[:])
            eng = nc.sync if ob % 2 else nc.gpsimd
            eng.dma_start(
                outT[128 * ob:128 * (ob + 1), NC * c:NC * (c + 1)], o_sb[:])

        def ffn_thunks(c):
            ts = [lambda c=c: ffn_load(c)]
            ts += [lambda c=c, fb=fb: ffn1(c, fb) for fb in range(NFB)]
            ts += [lambda c=c, ob=ob: ffn2(c, ob) for ob in range(DCH)]
            return ts

        # ---- the pipeline ----
        # AGs fire per 512-chunk, right after the chunk's attention output;
        # AG-dependent fillers are placed at least one full chunk later so
        # the in-order PE queue never head-of-line blocks on a collective.

        # filler schedules per self chunk (n_j = 4, 8, 12, 16).
        # Chunk c's attention carries chunk c+1's wemb DMA + projections as
        # fillers (PE work to hide the exp chain), plus the cross-side prep.
        # qc_proj(c) needs AG of self chunk c: schedule it at least one
        # full attention chunk after that AG fires, so the in-order PE
        # queue never head-of-line blocks on the collective.
        xq = {0: xc0}

        def next_proj(c):
            if c >= NSQ:
                return []
            ts = [lambda: xq.__setitem__(c, dma_wemb(c)) if c not in xq
                  else None]
            ts += [lambda: proj_q(c, xq[c]), lambda: proj_k(c, xq[c]),
                   lambda: proj_v(c, xq[c])]
            return ts

        def self_fillers(c):
            ts = next_proj(c + 1)
            if c == 2:
                ts += [lambda: proj_kc(0, xp0), lambda: proj_vc(0, xp0)]
            if c == 3:
                ts += [lambda: proj_kc(1, xp1), lambda: proj_vc(1, xp1),
                       lambda: qc_proj(0)]
            return ts

        xq[1] = dma_wemb(1)
        proj_q(0, xq[0])
        proj_k(0, xq[0])
        proj_v(0, xq[0])
        for c in range(NSQ):
            self_out = chkp.tile([128, NC], BF16, tag=f"oa{c % 2}",
                                 name=f"selfO{c}")
            attention_chunk(self_out[:],
                            qT[:, NC * c:NC * (c + 1)], kT, v65,
                            4 * (c + 1), causal_c=c, fillers=self_fillers(c))
            wd_c[c] = (allgather(self_out, f"w{c}"), 0)

        # cross chunks with qc/FFN fillers
        def cross_fillers(c):
            ts = []
            if c == 0:
                ts += [lambda: qc_proj(1)]
            if c == 1:
                ts += [lambda: qc_proj(2)]
            if c == 2:
                ts += [lambda: qc_proj(3)]
                ts += ffn_thunks(0)
            if c == 3:
                ts += ffn_thunks(1)
            return ts

        for c in range(NSQ):
            cross_out = chkp.tile([128, NC], BF16, tag=f"oa{c % 2}",
                                  name=f"crossO{c}")
            attention_chunk(cross_out[:],
                            qc_t[c][:], kcT, vc65, NSPB,
                            causal_c=None, fillers=cross_fillers(c))
            cd_c[c] = allgather(cross_out, f"c{c}")

        for th in ffn_thunks(NSQ - 2):
            th()
        for th in ffn_thunks(NSQ - 1):
            th()


_CACHED_NC = None


def _build():
    global _CACHED_NC
    if _CACHED_NC is None:
        nc = bacc.Bacc(
            "TRN2",
            target_bir_lowering=False,
            debug=False,
            num_devices=N_CORES,
        )
        with tile.TileContext(nc) as tc:
            decoder_kernel(tc)
        nc.compile()
        _CACHED_NC = nc
    return _CACHED_NC


def _pack_w(wT):
    """[1024, m] -> [128, 8*m]: d-chunk blocks side by side, partition-major."""
    m = wT.shape[1]
    return np.ascontiguousarray(
        wT.reshape(8, 128, m).transpose(1, 0, 2).reshape(128, 8 * m)
    ).astype(ml_dtypes.bfloat16)


def _pack_x(xT, nch):
    """[1024, nch*512] -> [128, nch * 8 * 512]: per seq-chunk c, the 8
    feature-blocks of that chunk's columns, contiguous."""
    return np.ascontiguousarray(
        xT.reshape(8, 128, nch, 512).transpose(1, 2, 0, 3)
        .reshape(128, nch * 8 * 512)).astype(ml_dtypes.bfloat16)


def make_in_maps(inputs):
    """Host-side prep: transposes + per-core weight slices + prepack."""
    f = np.ascontiguousarray
    wembT = _pack_x(np.asarray(inputs["wemb"], np.float32).T, NSQ)
    pembT = _pack_x(np.asarray(inputs["pemb"], np.float32).T, 2)
    in_maps = []
    for i in range(N_CORES):
        hsl = slice(HD * i, HD * (i + 1))
        fsl = slice(FF_SH * i, FF_SH * (i + 1))
        w2T = np.asarray(inputs["W2"], np.float32)[:, fsl].T  # [512, 1024]
        w2h = f(w2T.reshape(4, 128, 8, 128).transpose(1, 2, 0, 3)
                .reshape(128, 4096)).astype(ml_dtypes.bfloat16)
        in_maps.append({
            "wembT": wembT,
            "pembT": pembT,
            "wqmT": _pack_w(np.asarray(inputs["Wq_m"], np.float32)[hsl, :].T),
            "wkmT": _pack_w(np.asarray(inputs["Wk_m"], np.float32)[hsl, :].T),
            "wvmT": _pack_w(np.asarray(inputs["Wv_m"], np.float32)[hsl, :].T),
            "wqcT": _pack_w(np.asarray(inputs["Wq_c"], np.float32)[hsl, :].T),
            "wkcT": _pack_w(np.asarray(inputs["Wk_c"], np.float32)[hsl, :].T),
            "wvcT": _pack_w(np.asarray(inputs["Wv_c"], np.float32)[hsl, :].T),
            "w1T": _pack_w(np.asarray(inputs["W1"], np.float32)[fsl, :].T),
            "w2T": w2h,
        })
    return in_maps


def kernel(**inputs) -> np.ndarray:
    nc = _build()
    in_maps = make_in_maps(inputs)
    res = run_bass_kernel_spmd(nc, in_maps, core_ids=list(range(N_CORES)))
    acc = np.zeros((D_MODEL, S_W), dtype=np.float32)
    for i in range(N_CORES):
        acc += np.asarray(res.results[i]["outT"], np.float32)
    return np.ascontiguousarray(acc.T.astype(np.float32))
